# revision 1
# baseline (speedup 1.0000x reference)
"""Causal self-attention Trainium2 kernel (8-core SPMD).

Sharding: 8 cores = 4 batches x 2 head-groups (tensor parallel over heads).
Each core computes, for its batch b and its 8 heads:
  QKV projection (transposed layouts), causal flash-style attention without
  max-subtraction (scores are O(+-10), safe in fp32), and a partial output
  projection over its head-group's rows of W_proj.  The host sums the two
  partial outputs per batch (the "all-reduce" of the hint, done host-side).

Device layouts (per core):
  x       [T, C]    this batch's activations
  xT      [C, T]    built on-device via PE transposes (fp32 has no DMA xpose)
  Q^T,K^T [f, t]    f = head-major features (head pair per 128-chunk)
  V_ext   [t, 8*65] per head: 64 V columns + a ones column (softmax denom
                    falls out of the attn@V matmul for free)
  S^T     [k, q]    scores transposed; softmax denom = ones-row of V_ext
  y^T     [f, t]    normalized attention output, feeds W_proj matmul
  out     [T, C]    partial projection output (host adds the two halves)
"""

import numpy as np

import concourse.bass as bass
import concourse.mybir as mybir
import concourse.tile as tile
from concourse import bacc
from concourse.bass_utils import run_bass_kernel_spmd

F32 = mybir.dt.float32
P = 128
NEG = -1.0e30


def build_nc(T=2048, C=1024, n_loc_heads=8, debug=False):
    """Build the per-core SPMD program. T must be a multiple of 512."""
    D = 64
    HL = n_loc_heads              # local heads (8)
    FQK = HL * D                  # 512: Q (and K) features per core
    NQT = T // 512                # q-tiles of 512
    NTC = T // P                  # t-chunks of 128
    NCO = C // P                  # contraction chunks (8)
    NM = 2 * FQK // P             # Q+K feature chunks (8)
    NFC = FQK // P                # y^T feature chunks (4)
    NCT = C // 512                # output column tiles (2)
    Exp = mybir.ActivationFunctionType.Exp

    nc = bacc.Bacc(target_bir_lowering=False, debug=debug)
    x = nc.dram_tensor("x", [T, C], F32, kind="ExternalInput")
    wqk = nc.dram_tensor("wqk", [C, 2 * FQK], F32, kind="ExternalInput")
    wv = nc.dram_tensor("wv", [C, FQK], F32, kind="ExternalInput")
    wpr = nc.dram_tensor("wpr", [FQK, C], F32, kind="ExternalInput")
    bqk = nc.dram_tensor("bqk", [P, NM], F32, kind="ExternalInput")
    bv = nc.dram_tensor("bv", [P, FQK], F32, kind="ExternalInput")
    bpr = nc.dram_tensor("bpr", [P, C], F32, kind="ExternalInput")
    out = nc.dram_tensor("out", [T, C], F32, kind="ExternalOutput")

    with tile.TileContext(nc) as tc:
        with (
            tc.tile_pool(name="const", bufs=1) as cpool,
            tc.tile_pool(name="persist", bufs=1) as ppool,
            tc.tile_pool(name="xt", bufs=1) as xtp,
            tc.tile_pool(name="qt", bufs=2) as qtp,
            tc.tile_pool(name="yt", bufs=2) as ytp,
            tc.tile_pool(name="pt", bufs=3) as ptp,
            tc.tile_pool(name="wqk", bufs=6) as wqkp,
            tc.tile_pool(name="xin", bufs=2) as xinp,
            tc.tile_pool(name="oout", bufs=3) as outp,
            tc.tile_pool(name="dnm", bufs=3) as dnp,
            tc.tile_pool(name="dram", bufs=3, space="DRAM") as drp,
            tc.tile_pool(name="mm", bufs=4, space="PSUM") as mmp,
            tc.tile_pool(name="yps", bufs=2, space="PSUM") as ypp,
        ):
            # ---- constants ----
            ident = cpool.tile([P, P], F32, tag="ident")
            from concourse.masks import make_identity
            make_identity(nc, ident[:])

            # G[i, u] = 0 where u >= i + 384 else NEG; diag mask for shift s
            # is G[:, 384-s : 896-s]  (valid iff i + s <= j).
            G = cpool.tile([P, 896], F32, tag="gmask")
            nc.gpsimd.memset(G[:], 0.0)
            nc.gpsimd.affine_select(
                out=G[:], in_=G[:],
                compare_op=mybir.AluOpType.is_ge,
                fill=NEG, base=-384, channel_multiplier=-1,
                pattern=[[1, 896]],
            )

            bqk_sb = cpool.tile([P, NM], F32, tag="bqk")
            nc.sync.dma_start(bqk_sb[:], bqk[:, :])
            bv_sb = cpool.tile([P, FQK], F32, tag="bv")
            nc.sync.dma_start(bv_sb[:], bv[:, :])
            bpr_sb = cpool.tile([P, C], F32, tag="bpr")
            nc.sync.dma_start(bpr_sb[:], bpr[:, :])

            # ---- persistent tensors ----
            KT = ppool.tile([P, NFC, T], F32, tag="KT")
            VE = ppool.tile([P, NTC, HL * (D + 1)], F32, tag="VE")
            wv_sb = ppool.tile([P, NCO, FQK], F32, tag="wv")
            nc.sync.dma_start(
                wv_sb[:], wv.rearrange("(co ci) n -> ci co n", ci=P))
            wpr_sb = ppool.tile([P, NFC, C], F32, tag="wpr")
            nc.sync.dma_start(
                wpr_sb[:], wpr.rearrange("(fo fi) n -> fi fo n", fi=P))

            for qt in range(NQT):
                q0 = qt * 512
                # ---- phase A: transpose this q-tile of x into xT ----
                xTt = xtp.tile([P, NCO, 512], F32, tag="xT")
                for tc_i in range(4):
                    xrow = xinp.tile([P, C], F32, tag="xin")
                    nc.sync.dma_start(
                        xrow[:], x[q0 + tc_i * P: q0 + (tc_i + 1) * P, :])
                    for co in range(NCO):
                        tp = mmp.tile([P, P], F32, tag="mm")
                        nc.tensor.transpose(
                            tp[:], xrow[:, co * P:(co + 1) * P], ident[:])
                        dst = xTt[:, co, tc_i * P:(tc_i + 1) * P]
                        if co % 2 == 0:
                            nc.vector.tensor_copy(dst, tp[:])
                        else:
                            nc.scalar.copy(dst, tp[:])

                # ---- phase B: QKV projections for this q-tile ----
                QTt = qtp.tile([P, NFC, 512], F32, tag="QTt")
                for m in range(NM):
                    ps = mmp.tile([P, 512], F32, tag="mm")
                    for co in range(NCO):
                        wt = wqkp.tile([P, P], F32, tag="wqk")
                        nc.sync.dma_start(
                            wt[:], wqk[co * P:(co + 1) * P,
                                       m * P:(m + 1) * P])
                        nc.tensor.matmul(ps[:], wt[:], xTt[:, co, :],
                                         start=(co == 0), stop=(co == NCO - 1))
                    if m < NFC:
                        dst = QTt[:, m, :]
                    else:
                        dst = KT[:, m - NFC, q0:q0 + 512]
                    # dst = ps + b  (per-partition bias)
                    nc.scalar.add(dst, ps[:], bqk_sb[:, m:m + 1])

                for tc_i in range(4):
                    ps = mmp.tile([P, 512], F32, tag="mm")
                    for co in range(NCO):
                        nc.tensor.matmul(
                            ps[:], xTt[:, co, tc_i * P:(tc_i + 1) * P],
                            wv_sb[:, co, :],
                            start=(co == 0), stop=(co == NCO - 1))
                    tci = qt * 4 + tc_i
                    vev = VE[:, tci, :].rearrange("p (h e) -> p h e", e=D + 1)
                    nc.vector.tensor_add(
                        vev[:, :, :D],
                        ps[:].rearrange("p (h d) -> p h d", d=D),
                        bv_sb[:].rearrange("p (h d) -> p h d", d=D))
                    nc.vector.memset(vev[:, :, D:D + 1], 1.0)

                # ---- phase C: attention for this q-tile ----
                nk = 4 * (qt + 1)
                yTt = ytp.tile([P, NFC, 512], F32, tag="yTt")
                for h in range(HL):
                    po = 64 * (h % 2)
                    ch = h // 2
                    yps = ypp.tile([P, 512], F32, tag="yps")
                    for kc in range(nk):
                        sps = mmp.tile([P, 512], F32, tag="mm")
                        nc.tensor.matmul(
                            sps[:],
                            KT[po:po + 64, ch, kc * P:(kc + 1) * P],
                            QTt[po:po + 64, ch, :],
                            start=True, stop=True)
                        pt_t = ptp.tile([P, 512], F32, tag="pt")
                        if kc >= 4 * qt:
                            s = 128 * kc - 512 * qt
                            nc.vector.scalar_tensor_tensor(
                                pt_t[:], sps[:], 0.125,
                                G[:, 384 - s:896 - s],
                                mybir.AluOpType.mult, mybir.AluOpType.add)
                            nc.scalar.activation(pt_t[:], pt_t[:], Exp)
                        else:
                            nc.scalar.activation(pt_t[:], sps[:], Exp,
                                                 scale=0.125)
                        nc.tensor.matmul(
                            yps[:D + 1, :],
                            VE[:, kc, h * (D + 1):(h + 1) * (D + 1)],
                            pt_t[:],
                            start=(kc == 0), stop=(kc == nk - 1))
                    rd = dnp.tile([1, 512], F32, tag="rd")
                    nc.vector.reciprocal(rd[:], yps[D:D + 1, :])
                    dsc = drp.tile([1, 512], F32, tag="dsc")
                    nc.sync.dma_start(dsc[:], rd[:])
                    rep = dnp.tile([64, 512], F32, tag="rep")
                    nc.sync.dma_start(rep[:], dsc[:].to_broadcast((64, 512)))
                    nc.vector.tensor_mul(
                        yTt[po:po + 64, ch, :], yps[:D, :], rep[:])

                # ---- phase D: output projection for this q-tile ----
                for tc_i in range(4):
                    for ct in range(NCT):
                        ps = mmp.tile([P, 512], F32, tag="mm")
                        for fc in range(NFC):
                            nc.tensor.matmul(
                                ps[:],
                                yTt[:, fc, tc_i * P:(tc_i + 1) * P],
                                wpr_sb[:, fc, ct * 512:(ct + 1) * 512],
                                start=(fc == 0), stop=(fc == NFC - 1))
                        ot = outp.tile([P, 512], F32, tag="oout")
                        nc.vector.tensor_add(
                            ot[:], ps[:], bpr_sb[:, ct * 512:(ct + 1) * 512])
                        nc.sync.dma_start(
                            out[q0 + tc_i * P:q0 + (tc_i + 1) * P,
                                ct * 512:(ct + 1) * 512], ot[:])

    nc.compile()
    return nc


_CACHE = {}


def _get_nc():
    if "nc" not in _CACHE:
        _CACHE["nc"] = build_nc()
    return _CACHE["nc"]


def make_in_maps(x, W_attn, b_attn, W_proj, b_proj, B=4, C=1024):
    x = np.ascontiguousarray(np.asarray(x, dtype=np.float32))
    W_attn = np.asarray(W_attn, dtype=np.float32)
    b_attn = np.asarray(b_attn, dtype=np.float32)
    W_proj = np.asarray(W_proj, dtype=np.float32)
    b_proj = np.asarray(b_proj, dtype=np.float32)
    in_maps = []
    for core in range(2 * B):
        b, hg = core // 2, core % 2
        s = slice(hg * 512, (hg + 1) * 512)
        wqk_c = np.ascontiguousarray(
            np.concatenate([W_attn[:, s], W_attn[:, C + hg * 512:
                                                 C + (hg + 1) * 512]], axis=1))
        wv_c = np.ascontiguousarray(W_attn[:, 2 * C + hg * 512:
                                           2 * C + (hg + 1) * 512])
        wpr_c = np.ascontiguousarray(W_proj[hg * 512:(hg + 1) * 512, :])
        bqk_vec = np.concatenate([b_attn[s], b_attn[C + hg * 512:
                                                    C + (hg + 1) * 512]])
        bqk_c = np.ascontiguousarray(bqk_vec.reshape(8, 128).T)
        bv_c = np.ascontiguousarray(
            np.tile(b_attn[2 * C + hg * 512:2 * C + (hg + 1) * 512][None, :],
                    (128, 1)))
        if hg == 0:
            bpr_c = np.ascontiguousarray(np.tile(b_proj[None, :], (128, 1)))
        else:
            bpr_c = np.zeros((128, C), dtype=np.float32)
        in_maps.append({
            "x": np.ascontiguousarray(x[b]),
            "wqk": wqk_c, "wv": wv_c, "wpr": wpr_c,
            "bqk": bqk_c, "bv": bv_c, "bpr": bpr_c,
        })
    return in_maps


def kernel(x, W_attn, b_attn, W_proj, b_proj):
    B, T, C = 4, 2048, 1024
    nc = _get_nc()
    in_maps = make_in_maps(x, W_attn, b_attn, W_proj, b_proj, B=B, C=C)
    res = run_bass_kernel_spmd(nc, in_maps, list(range(2 * B)))
    out = np.empty((B, T, C), dtype=np.float32)
    for b in range(B):
        out[b] = res.results[2 * b]["out"] + res.results[2 * b + 1]["out"]
    return out


# revision 3
# speedup vs baseline: 1.0559x; 1.0559x over previous
"""Causal self-attention Trainium2 kernel (8-core SPMD).

Sharding: 8 cores = 4 batches x 2 head-groups (tensor parallel over heads).
Each core computes, for its batch b and its 8 heads:
  QKV projection (transposed layouts), causal flash-style attention without
  max-subtraction (scores are O(+-10), safe in fp32), and a partial output
  projection over its head-group's rows of W_proj.  The host sums the two
  partial outputs per batch (the "all-reduce" of the hint, done host-side).

Device layouts (per core):
  x       [T, C]    this batch's activations
  xT      [C, T]    built on-device via PE transposes (fp32 has no DMA xpose)
  Q^T,K^T [f, t]    f = head-major features (head pair per 128-chunk)
  V_ext   [t, 8*65] per head: 64 V columns + a ones column (softmax denom
                    falls out of the attn@V matmul for free)
  S^T     [k, q]    scores transposed; softmax denom = ones-row of V_ext
  y^T     [f, t]    normalized attention output, feeds W_proj matmul
  out     [T, C]    partial projection output (host adds the two halves)
"""

import numpy as np

import concourse.bass as bass
import concourse.mybir as mybir
import concourse.tile as tile
from concourse import bacc
from concourse.bass_utils import run_bass_kernel_spmd

F32 = mybir.dt.float32
P = 128
NEG = -1.0e30


def build_nc(T=2048, C=1024, n_loc_heads=8, debug=False):
    """Build the per-core SPMD program. T must be a multiple of 512."""
    D = 64
    HL = n_loc_heads              # local heads (8)
    FQK = HL * D                  # 512: Q (and K) features per core
    NQT = T // 512                # q-tiles of 512
    NTC = T // P                  # t-chunks of 128
    NCO = C // P                  # contraction chunks (8)
    NM = 2 * FQK // P             # Q+K feature chunks (8)
    NFC = FQK // P                # y^T feature chunks (4)
    NCT = C // 512                # output column tiles (2)
    Exp = mybir.ActivationFunctionType.Exp

    nc = bacc.Bacc(target_bir_lowering=False, debug=debug)
    x = nc.dram_tensor("x", [T, C], F32, kind="ExternalInput")
    wqk = nc.dram_tensor("wqk", [C, 2 * FQK], F32, kind="ExternalInput")
    wv = nc.dram_tensor("wv", [C, FQK], F32, kind="ExternalInput")
    wpr = nc.dram_tensor("wpr", [FQK, C], F32, kind="ExternalInput")
    bqk = nc.dram_tensor("bqk", [P, NM], F32, kind="ExternalInput")
    bv = nc.dram_tensor("bv", [P, FQK], F32, kind="ExternalInput")
    bpr = nc.dram_tensor("bpr", [P, C], F32, kind="ExternalInput")
    out = nc.dram_tensor("out", [T, C], F32, kind="ExternalOutput")

    with tile.TileContext(nc) as tc:
        with (
            tc.tile_pool(name="const", bufs=1) as cpool,
            tc.tile_pool(name="persist", bufs=1) as ppool,
            tc.tile_pool(name="xt", bufs=1) as xtp,
            tc.tile_pool(name="qt", bufs=1) as qtp,
            tc.tile_pool(name="yt", bufs=2) as ytp,
            tc.tile_pool(name="pt", bufs=2) as ptp,
            tc.tile_pool(name="yx", bufs=4) as yxp,
            tc.tile_pool(name="wqk", bufs=4) as wqkp,
            tc.tile_pool(name="xin", bufs=2) as xinp,
            tc.tile_pool(name="oout", bufs=2) as outp,
            tc.tile_pool(name="dnm", bufs=2) as dnp,
            tc.tile_pool(name="dram", bufs=3, space="DRAM") as drp,
            tc.tile_pool(name="mm", bufs=2, space="PSUM") as mmp,
            tc.tile_pool(name="sp", bufs=2, space="PSUM") as spp,
            tc.tile_pool(name="yps", bufs=2, space="PSUM") as ypp,
        ):
            # ---- constants ----
            ident = cpool.tile([P, P], F32, tag="ident")
            from concourse.masks import make_identity
            make_identity(nc, ident[:])

            # G[i, u] = 0 where u >= i + 384 else NEG; diag mask for shift s
            # is G[:, 384-s : 896-s]  (valid iff i + s <= j).
            G = cpool.tile([P, 896], F32, tag="gmask")
            nc.gpsimd.memset(G[:], 0.0)
            nc.gpsimd.affine_select(
                out=G[:], in_=G[:],
                compare_op=mybir.AluOpType.is_ge,
                fill=NEG, base=-384, channel_multiplier=-1,
                pattern=[[1, 896]],
            )

            bqk_sb = cpool.tile([P, NM], F32, tag="bqk")
            nc.sync.dma_start(bqk_sb[:], bqk[:, :])
            bv_sb = cpool.tile([P, FQK], F32, tag="bv")
            nc.sync.dma_start(bv_sb[:], bv[:, :])
            bpr_sb = cpool.tile([P, C], F32, tag="bpr")
            nc.sync.dma_start(bpr_sb[:], bpr[:, :])

            # ---- persistent tensors ----
            KT = ppool.tile([P, NFC, T], F32, tag="KT")
            VE = ppool.tile([P, NTC, HL * (D + 1)], F32, tag="VE")
            wv_sb = ppool.tile([P, NCO, FQK], F32, tag="wv")
            nc.sync.dma_start(
                wv_sb[:], wv.rearrange("(co ci) n -> ci co n", ci=P))
            wpr_sb = ppool.tile([P, NFC, C], F32, tag="wpr")
            nc.sync.dma_start(
                wpr_sb[:], wpr.rearrange("(fo fi) n -> fi fo n", fi=P))

            for qt in range(NQT):
                q0 = qt * 512
                # ---- phase A: transpose this q-tile of x into xT ----
                xTt = xtp.tile([P, NCO, 512], F32, tag="xT")
                for tc_i in range(4):
                    xrow = xinp.tile([P, C], F32, tag="xin")
                    nc.sync.dma_start(
                        xrow[:], x[q0 + tc_i * P: q0 + (tc_i + 1) * P, :])
                    for co in range(NCO):
                        tp = mmp.tile([P, P], F32, tag="mm")
                        nc.tensor.transpose(
                            tp[:], xrow[:, co * P:(co + 1) * P], ident[:])
                        dst = xTt[:, co, tc_i * P:(tc_i + 1) * P]
                        if co % 2 == 0:
                            nc.vector.tensor_copy(dst, tp[:])
                        else:
                            nc.scalar.copy(dst, tp[:])

                # ---- phase B: QKV projections for this q-tile ----
                QTt = qtp.tile([P, NFC, 512], F32, tag="QTt")
                for m in range(NM):
                    ps = mmp.tile([P, 512], F32, tag="mm")
                    for co in range(NCO):
                        wt = wqkp.tile([P, P], F32, tag="wqk")
                        nc.sync.dma_start(
                            wt[:], wqk[co * P:(co + 1) * P,
                                       m * P:(m + 1) * P])
                        nc.tensor.matmul(ps[:], wt[:], xTt[:, co, :],
                                         start=(co == 0), stop=(co == NCO - 1))
                    if m < NFC:
                        dst = QTt[:, m, :]
                    else:
                        dst = KT[:, m - NFC, q0:q0 + 512]
                    # dst = ps + b  (per-partition bias)
                    nc.scalar.add(dst, ps[:], bqk_sb[:, m:m + 1])

                for tc_i in range(4):
                    ps = mmp.tile([P, 512], F32, tag="mm")
                    for co in range(NCO):
                        nc.tensor.matmul(
                            ps[:], xTt[:, co, tc_i * P:(tc_i + 1) * P],
                            wv_sb[:, co, :],
                            start=(co == 0), stop=(co == NCO - 1))
                    tci = qt * 4 + tc_i
                    vev = VE[:, tci, :].rearrange("p (h e) -> p h e", e=D + 1)
                    nc.vector.tensor_add(
                        vev[:, :, :D],
                        ps[:].rearrange("p (h d) -> p h d", d=D),
                        bv_sb[:].rearrange("p (h d) -> p h d", d=D))
                    nc.vector.memset(vev[:, :, D:D + 1], 1.0)

                # ---- phase C: attention for this q-tile (head pairs) ----
                nk = 4 * (qt + 1)
                yTt = ytp.tile([P, NFC, 512], F32, tag="yTt")
                for ch in range(NFC):            # head pair (2ch, 2ch+1)
                    ypsA = ypp.tile([P, 512], F32, tag="yps")
                    ypsB = ypp.tile([P, 512], F32, tag="yps")
                    for kc in range(nk):
                        # S^T for both heads of the pair, packed in the PE
                        # array via row tiling (each head contracts over 64
                        # partitions), into one 2-bank psum tile.
                        sp2 = spp.tile([P, 1024], F32, tag="sp")
                        nc.tensor.matmul(
                            sp2[:, 0:512],
                            KT[0:64, ch, kc * P:(kc + 1) * P],
                            QTt[0:64, ch, :],
                            start=True, stop=True, tile_position=(0, 0))
                        nc.tensor.matmul(
                            sp2[:, 512:1024],
                            KT[64:128, ch, kc * P:(kc + 1) * P],
                            QTt[64:128, ch, :],
                            start=True, stop=True, tile_position=(64, 0))
                        pt_t = ptp.tile([P, 1024], F32, tag="pt")
                        if kc >= 4 * qt:
                            s = 128 * kc - 512 * qt
                            nc.vector.scalar_tensor_tensor(
                                pt_t[:].rearrange("p (h q) -> p h q", h=2),
                                sp2[:].rearrange("p (h q) -> p h q", h=2),
                                0.125,
                                G[:, None, 384 - s:896 - s].to_broadcast(
                                    (P, 2, 512)),
                                mybir.AluOpType.mult, mybir.AluOpType.add)
                            nc.scalar.activation(pt_t[:], pt_t[:], Exp)
                        else:
                            nc.scalar.activation(pt_t[:], sp2[:], Exp,
                                                 scale=0.125)
                        hA, hB = 2 * ch, 2 * ch + 1
                        nc.tensor.matmul(
                            ypsA[:D + 1, :],
                            VE[:, kc, hA * (D + 1):(hA + 1) * (D + 1)],
                            pt_t[:, 0:512],
                            start=(kc == 0), stop=(kc == nk - 1))
                        nc.tensor.matmul(
                            ypsB[:D + 1, :],
                            VE[:, kc, hB * (D + 1):(hB + 1) * (D + 1)],
                            pt_t[:, 512:1024],
                            start=(kc == 0), stop=(kc == nk - 1))
                    for po, yps in ((0, ypsA), (64, ypsB)):
                        yext = yxp.tile([D + 1, 512], F32, tag="yext")
                        nc.vector.tensor_copy(yext[:], yps[:D + 1, :])
                        rd = dnp.tile([1, 512], F32, tag="rd")
                        nc.vector.reciprocal(rd[:], yext[D:D + 1, :])
                        dsc = drp.tile([1, 512], F32, tag="dsc")
                        nc.sync.dma_start(dsc[:], rd[:])
                        rep = dnp.tile([64, 512], F32, tag="rep")
                        nc.sync.dma_start(rep[:],
                                          dsc[:].to_broadcast((64, 512)))
                        nc.vector.tensor_mul(
                            yTt[po:po + 64, ch, :], yext[:D, :], rep[:])

                # ---- phase D: output projection for this q-tile ----
                for tc_i in range(4):
                    for ct in range(NCT):
                        ps = mmp.tile([P, 512], F32, tag="mm")
                        for fc in range(NFC):
                            nc.tensor.matmul(
                                ps[:],
                                yTt[:, fc, tc_i * P:(tc_i + 1) * P],
                                wpr_sb[:, fc, ct * 512:(ct + 1) * 512],
                                start=(fc == 0), stop=(fc == NFC - 1))
                        ot = outp.tile([P, 512], F32, tag="oout")
                        nc.vector.tensor_add(
                            ot[:], ps[:], bpr_sb[:, ct * 512:(ct + 1) * 512])
                        nc.sync.dma_start(
                            out[q0 + tc_i * P:q0 + (tc_i + 1) * P,
                                ct * 512:(ct + 1) * 512], ot[:])

    nc.compile()
    return nc


_CACHE = {}


def _get_nc():
    if "nc" not in _CACHE:
        _CACHE["nc"] = build_nc()
    return _CACHE["nc"]


def make_in_maps(x, W_attn, b_attn, W_proj, b_proj, B=4, C=1024):
    x = np.ascontiguousarray(np.asarray(x, dtype=np.float32))
    W_attn = np.asarray(W_attn, dtype=np.float32)
    b_attn = np.asarray(b_attn, dtype=np.float32)
    W_proj = np.asarray(W_proj, dtype=np.float32)
    b_proj = np.asarray(b_proj, dtype=np.float32)
    in_maps = []
    for core in range(2 * B):
        b, hg = core // 2, core % 2
        s = slice(hg * 512, (hg + 1) * 512)
        wqk_c = np.ascontiguousarray(
            np.concatenate([W_attn[:, s], W_attn[:, C + hg * 512:
                                                 C + (hg + 1) * 512]], axis=1))
        wv_c = np.ascontiguousarray(W_attn[:, 2 * C + hg * 512:
                                           2 * C + (hg + 1) * 512])
        wpr_c = np.ascontiguousarray(W_proj[hg * 512:(hg + 1) * 512, :])
        bqk_vec = np.concatenate([b_attn[s], b_attn[C + hg * 512:
                                                    C + (hg + 1) * 512]])
        bqk_c = np.ascontiguousarray(bqk_vec.reshape(8, 128).T)
        bv_c = np.ascontiguousarray(
            np.tile(b_attn[2 * C + hg * 512:2 * C + (hg + 1) * 512][None, :],
                    (128, 1)))
        if hg == 0:
            bpr_c = np.ascontiguousarray(np.tile(b_proj[None, :], (128, 1)))
        else:
            bpr_c = np.zeros((128, C), dtype=np.float32)
        in_maps.append({
            "x": np.ascontiguousarray(x[b]),
            "wqk": wqk_c, "wv": wv_c, "wpr": wpr_c,
            "bqk": bqk_c, "bv": bv_c, "bpr": bpr_c,
        })
    return in_maps


def kernel(x, W_attn, b_attn, W_proj, b_proj):
    B, T, C = 4, 2048, 1024
    nc = _get_nc()
    in_maps = make_in_maps(x, W_attn, b_attn, W_proj, b_proj, B=B, C=C)
    res = run_bass_kernel_spmd(nc, in_maps, list(range(2 * B)))
    out = np.empty((B, T, C), dtype=np.float32)
    for b in range(B):
        out[b] = res.results[2 * b]["out"] + res.results[2 * b + 1]["out"]
    return out


# revision 9
# speedup vs baseline: 1.0828x; 1.0255x over previous
"""Causal self-attention Trainium2 kernel (8-core SPMD).

Sharding: 8 cores = 4 batches x 2 head-groups (tensor parallel over heads).
Each core computes, for its batch b and its 8 heads:
  QKV projection (transposed layouts), causal flash-style attention without
  max-subtraction (scores are O(+-10), safe in fp32), and a partial output
  projection over its head-group's rows of W_proj.  The host sums the two
  partial outputs per batch (the "all-reduce" of the hint, done host-side).

Device layouts (per core):
  x       [T, C]    this batch's activations
  xT      [C, T]    built on-device via PE transposes (fp32 has no DMA xpose)
  Q^T,K^T [f, t]    f = head-major features (head pair per 128-chunk)
  V_ext   [t, 8*65] per head: 64 V columns + a ones column (softmax denom
                    falls out of the attn@V matmul for free)
  S^T     [k, q]    scores transposed; softmax denom = ones-row of V_ext
  y^T     [f, t]    normalized attention output, feeds W_proj matmul
  out     [T, C]    partial projection output (host adds the two halves)
"""

import numpy as np

import concourse.bass as bass
import concourse.mybir as mybir
import concourse.tile as tile
from concourse import bacc
from concourse.bass_utils import run_bass_kernel_spmd

F32 = mybir.dt.float32
P = 128
NEG = -1.0e30


def build_nc(T=2048, C=1024, n_loc_heads=8, debug=False, reps=1,
             mm_dt=mybir.dt.float32r):
    """Build the per-core SPMD program. T must be a multiple of 512."""
    D = 64
    HL = n_loc_heads              # local heads (8)
    FQK = HL * D                  # 512: Q (and K) features per core
    NQT = T // 512                # q-tiles of 512
    NTC = T // P                  # t-chunks of 128
    NCO = C // P                  # contraction chunks (8)
    NM = 2 * FQK // P             # Q+K feature chunks (8)
    NFC = FQK // P                # y^T feature chunks (4)
    NCT = C // 512                # output column tiles (2)
    Exp = mybir.ActivationFunctionType.Exp
    r = lambda ap: ap
    MDT = mm_dt

    nc = bacc.Bacc(target_bir_lowering=False, debug=debug)
    x = nc.dram_tensor("x", [T, C], F32, kind="ExternalInput")
    wqk = nc.dram_tensor("wqk", [C, 2 * FQK], mm_dt, kind="ExternalInput")
    wv = nc.dram_tensor("wv", [C, FQK], mm_dt, kind="ExternalInput")
    wpr = nc.dram_tensor("wpr", [FQK, C], mm_dt, kind="ExternalInput")
    bqk = nc.dram_tensor("bqk", [P, NM], F32, kind="ExternalInput")
    bv = nc.dram_tensor("bv", [P, FQK], F32, kind="ExternalInput")
    bpr = nc.dram_tensor("bpr", [P, C], F32, kind="ExternalInput")
    out = nc.dram_tensor("out", [T, C], F32, kind="ExternalOutput")

    with tile.TileContext(nc) as tc:
        with (
            tc.tile_pool(name="const", bufs=1) as cpool,
            tc.tile_pool(name="persist", bufs=1) as ppool,
            tc.tile_pool(name="xt", bufs=1) as xtp,
            tc.tile_pool(name="qt", bufs=1) as qtp,
            tc.tile_pool(name="yt", bufs=2) as ytp,
            tc.tile_pool(name="pt", bufs=2) as ptp,
            tc.tile_pool(name="yx", bufs=4) as yxp,
            tc.tile_pool(name="wqk", bufs=4) as wqkp,
            tc.tile_pool(name="xin", bufs=2) as xinp,
            tc.tile_pool(name="oout", bufs=2) as outp,
            tc.tile_pool(name="dnm", bufs=2) as dnp,
            tc.tile_pool(name="dram", bufs=3, space="DRAM") as drp,
            tc.tile_pool(name="mm", bufs=2, space="PSUM") as mmp,
            tc.tile_pool(name="sp", bufs=2, space="PSUM") as spp,
            tc.tile_pool(name="yps", bufs=2, space="PSUM") as ypp,
        ):
            # ---- constants ----
            ident = cpool.tile([P, P], F32, tag="ident")
            from concourse.masks import make_identity
            make_identity(nc, ident[:])

            # G[i, u] = 0 where u >= i + 384 else NEG; diag mask for shift s
            # is G[:, 384-s : 896-s]  (valid iff i + s <= j).
            G = cpool.tile([P, 896], F32, tag="gmask")
            nc.gpsimd.memset(G[:], 0.0)
            nc.gpsimd.affine_select(
                out=G[:], in_=G[:],
                compare_op=mybir.AluOpType.is_ge,
                fill=NEG, base=-384, channel_multiplier=-1,
                pattern=[[1, 896]],
            )

            ones_sb = cpool.tile([P, HL, 1], F32, tag="ones")
            nc.vector.memset(ones_sb[:], 1.0)
            bqk_sb = cpool.tile([P, NM], F32, tag="bqk")
            nc.sync.dma_start(bqk_sb[:], bqk[:, :])
            bv_sb = cpool.tile([P, FQK], F32, tag="bv")
            nc.sync.dma_start(bv_sb[:], bv[:, :])
            bpr_sb = cpool.tile([P, C], F32, tag="bpr")
            nc.sync.dma_start(bpr_sb[:], bpr[:, :])

            # ---- persistent tensors ----
            KT = ppool.tile([P, NFC, T], MDT, tag="KT")
            VE = ppool.tile([P, NTC, HL * (D + 1)], MDT, tag="VE")
            wv_sb = ppool.tile([P, NCO, FQK], MDT, tag="wv")
            nc.sync.dma_start(
                wv_sb[:], wv.rearrange("(co ci) n -> ci co n", ci=P))
            wpr_sb = ppool.tile([P, NFC, C], MDT, tag="wpr")
            nc.sync.dma_start(
                wpr_sb[:], wpr.rearrange("(fo fi) n -> fi fo n", fi=P))

            for rep in range(reps):
              for qt in range(NQT):
                q0 = qt * 512
                # ---- phase A: transpose this q-tile of x into xT ----
                xTt = xtp.tile([P, NCO, 512], MDT, tag="xT")
                for tc_i in range(4):
                    xrow = xinp.tile([P, C], F32, tag="xin")
                    nc.sync.dma_start(
                        xrow[:], x[q0 + tc_i * P: q0 + (tc_i + 1) * P, :])
                    for co in range(NCO):
                        tp = mmp.tile([P, P], F32, tag="mm")
                        nc.tensor.transpose(
                            r(tp[:]), r(xrow[:, co * P:(co + 1) * P]),
                            r(ident[:]))
                        dst = xTt[:, co, tc_i * P:(tc_i + 1) * P]
                        if co % 2 == 0:
                            nc.vector.tensor_copy(dst, tp[:])
                        else:
                            nc.scalar.copy(dst, tp[:])

                # ---- phase B: QKV projections for this q-tile ----
                QTt = qtp.tile([P, NFC, 512], MDT, tag="QTt")
                for m in range(NM):
                    ps = mmp.tile([P, 512], F32, tag="mm")
                    for co in range(NCO):
                        wt = wqkp.tile([P, P], MDT, tag="wqk")
                        nc.sync.dma_start(
                            wt[:], wqk[co * P:(co + 1) * P,
                                       m * P:(m + 1) * P])
                        nc.tensor.matmul(ps[:], r(wt[:]), r(xTt[:, co, :]),
                                         start=(co == 0), stop=(co == NCO - 1))
                    if m < NFC:
                        dst = QTt[:, m, :]
                    else:
                        dst = KT[:, m - NFC, q0:q0 + 512]
                    # dst = ps + b  (per-partition bias)
                    nc.scalar.add(dst, ps[:], bqk_sb[:, m:m + 1])

                for tc_i in range(4):
                    ps = mmp.tile([P, 512], F32, tag="mm")
                    for co in range(NCO):
                        nc.tensor.matmul(
                            ps[:], r(xTt[:, co, tc_i * P:(tc_i + 1) * P]),
                            r(wv_sb[:, co, :]),
                            start=(co == 0), stop=(co == NCO - 1))
                    tci = qt * 4 + tc_i
                    vev = VE[:, tci, :].rearrange("p (h e) -> p h e", e=D + 1)
                    nc.vector.tensor_add(
                        vev[:, :, :D],
                        ps[:].rearrange("p (h d) -> p h d", d=D),
                        bv_sb[:].rearrange("p (h d) -> p h d", d=D))
                    nc.vector.tensor_copy(vev[:, :, D:D + 1], ones_sb[:])

                # ---- phase C: attention for this q-tile (head pairs) ----
                nk = 4 * (qt + 1)
                yTt = ytp.tile([P, NFC, 512], MDT, tag="yTt")
                for ch in range(NFC):            # head pair (2ch, 2ch+1)
                    ypsA = ypp.tile([P, 512], F32, tag="yps")
                    ypsB = ypp.tile([P, 512], F32, tag="yps")
                    for kc in range(nk):
                        # S^T for both heads of the pair, packed in the PE
                        # array via row tiling (each head contracts over 64
                        # partitions), into one 2-bank psum tile.
                        sp2 = spp.tile([P, 1024], F32, tag="sp")
                        nc.tensor.matmul(
                            sp2[:, 0:512],
                            r(KT[0:64, ch, kc * P:(kc + 1) * P]),
                            r(QTt[0:64, ch, :]),
                            start=True, stop=True, tile_position=(0, 0))
                        nc.tensor.matmul(
                            sp2[:, 512:1024],
                            r(KT[64:128, ch, kc * P:(kc + 1) * P]),
                            r(QTt[64:128, ch, :]),
                            start=True, stop=True, tile_position=(64, 0))
                        pt_t = ptp.tile([P, 1024], MDT, tag="pt")
                        if kc >= 4 * qt:
                            s = 128 * kc - 512 * qt
                            nc.vector.scalar_tensor_tensor(
                                pt_t[:].rearrange("p (h q) -> p h q", h=2),
                                sp2[:].rearrange("p (h q) -> p h q", h=2),
                                0.125,
                                G[:, None, 384 - s:896 - s].to_broadcast(
                                    (P, 2, 512)),
                                mybir.AluOpType.mult, mybir.AluOpType.add)
                            nc.scalar.activation(pt_t[:], pt_t[:], Exp)
                        else:
                            nc.scalar.activation(pt_t[:], sp2[:], Exp,
                                                 scale=0.125)
                        hA, hB = 2 * ch, 2 * ch + 1
                        nc.tensor.matmul(
                            ypsA[:D + 1, :],
                            r(VE[:, kc, hA * (D + 1):(hA + 1) * (D + 1)]),
                            r(pt_t[:, 0:512]),
                            start=(kc == 0), stop=(kc == nk - 1))
                        nc.tensor.matmul(
                            ypsB[:D + 1, :],
                            r(VE[:, kc, hB * (D + 1):(hB + 1) * (D + 1)]),
                            r(pt_t[:, 512:1024]),
                            start=(kc == 0), stop=(kc == nk - 1))
                    for po, yps in ((0, ypsA), (64, ypsB)):
                        yext = yxp.tile([D + 1, 512], F32, tag="yext")
                        nc.vector.tensor_copy(yext[:], yps[:D + 1, :])
                        rd = dnp.tile([1, 512], F32, tag="rd")
                        nc.vector.reciprocal(rd[:], yext[D:D + 1, :])
                        dsc = drp.tile([1, 512], F32, tag="dsc")
                        nc.sync.dma_start(dsc[:], rd[:])
                        rep = dnp.tile([64, 512], F32, tag="rep")
                        nc.sync.dma_start(rep[:],
                                          dsc[:].to_broadcast((64, 512)))
                        nc.vector.tensor_mul(
                            yTt[po:po + 64, ch, :], yext[:D, :], rep[:])

                # ---- phase D: output projection for this q-tile ----
                for tc_i in range(4):
                    for ct in range(NCT):
                        ps = mmp.tile([P, 512], F32, tag="mm")
                        for fc in range(NFC):
                            nc.tensor.matmul(
                                ps[:],
                                r(yTt[:, fc, tc_i * P:(tc_i + 1) * P]),
                                r(wpr_sb[:, fc, ct * 512:(ct + 1) * 512]),
                                start=(fc == 0), stop=(fc == NFC - 1))
                        ot = outp.tile([P, 512], F32, tag="oout")
                        nc.vector.tensor_add(
                            ot[:], ps[:], bpr_sb[:, ct * 512:(ct + 1) * 512])
                        nc.sync.dma_start(
                            out[q0 + tc_i * P:q0 + (tc_i + 1) * P,
                                ct * 512:(ct + 1) * 512], ot[:])

    nc.compile()
    return nc


_CACHE = {}


def _get_nc():
    if "nc" not in _CACHE:
        _CACHE["nc"] = build_nc()
    return _CACHE["nc"]


def make_in_maps(x, W_attn, b_attn, W_proj, b_proj, B=4, C=1024):
    x = np.ascontiguousarray(np.asarray(x, dtype=np.float32))
    W_attn = np.asarray(W_attn, dtype=np.float32)
    b_attn = np.asarray(b_attn, dtype=np.float32)
    W_proj = np.asarray(W_proj, dtype=np.float32)
    b_proj = np.asarray(b_proj, dtype=np.float32)
    in_maps = []
    for core in range(2 * B):
        b, hg = core // 2, core % 2
        s = slice(hg * 512, (hg + 1) * 512)
        wqk_c = np.ascontiguousarray(
            np.concatenate([W_attn[:, s], W_attn[:, C + hg * 512:
                                                 C + (hg + 1) * 512]], axis=1))
        wv_c = np.ascontiguousarray(W_attn[:, 2 * C + hg * 512:
                                           2 * C + (hg + 1) * 512])
        wpr_c = np.ascontiguousarray(W_proj[hg * 512:(hg + 1) * 512, :])
        bqk_vec = np.concatenate([b_attn[s], b_attn[C + hg * 512:
                                                    C + (hg + 1) * 512]])
        bqk_c = np.ascontiguousarray(bqk_vec.reshape(8, 128).T)
        bv_c = np.ascontiguousarray(
            np.tile(b_attn[2 * C + hg * 512:2 * C + (hg + 1) * 512][None, :],
                    (128, 1)))
        if hg == 0:
            bpr_c = np.ascontiguousarray(np.tile(b_proj[None, :], (128, 1)))
        else:
            bpr_c = np.zeros((128, C), dtype=np.float32)
        in_maps.append({
            "x": np.ascontiguousarray(x[b]),
            "wqk": wqk_c, "wv": wv_c, "wpr": wpr_c,
            "bqk": bqk_c, "bv": bv_c, "bpr": bpr_c,
        })
    return in_maps


def kernel(x, W_attn, b_attn, W_proj, b_proj):
    B, T, C = 4, 2048, 1024
    nc = _get_nc()
    in_maps = make_in_maps(x, W_attn, b_attn, W_proj, b_proj, B=B, C=C)
    res = run_bass_kernel_spmd(nc, in_maps, list(range(2 * B)))
    out = np.empty((B, T, C), dtype=np.float32)
    for b in range(B):
        out[b] = res.results[2 * b]["out"] + res.results[2 * b + 1]["out"]
    return out


# revision 11
# speedup vs baseline: 1.1873x; 1.0965x over previous
"""Causal self-attention Trainium2 kernel (8-core SPMD).

Sharding: 8 cores = 4 batches x 2 head-groups (tensor parallel over heads).
Each core computes, for its batch b and its 8 heads:
  QKV projection (transposed layouts), causal flash-style attention without
  max-subtraction (scores are O(+-10), safe in fp32), and a partial output
  projection over its head-group's rows of W_proj.  The host sums the two
  partial outputs per batch (the "all-reduce" of the hint, done host-side).

Device layouts (per core):
  x       [T, C]    this batch's activations
  xT      [C, T]    built on-device via PE transposes (fp32 has no DMA xpose)
  Q^T,K^T [f, t]    f = head-major features (head pair per 128-chunk)
  V_ext   [t, 8*65] per head: 64 V columns + a ones column (softmax denom
                    falls out of the attn@V matmul for free)
  S^T     [k, q]    scores transposed; softmax denom = ones-row of V_ext
  y^T     [f, t]    normalized attention output, feeds W_proj matmul
  out     [T, C]    partial projection output (host adds the two halves)
"""

import numpy as np

import concourse.bass as bass
import concourse.mybir as mybir
import concourse.tile as tile
from concourse import bacc
from concourse.bass_utils import run_bass_kernel_spmd

F32 = mybir.dt.float32
P = 128
NEG = -1.0e30


def build_nc(T=2048, C=1024, n_loc_heads=8, debug=False, reps=1,
             mm_dt=mybir.dt.float32r, gsel=True):
    """Build the per-core SPMD program. T must be a multiple of 512."""
    D = 64
    HL = n_loc_heads              # local heads (8)
    FQK = HL * D                  # 512: Q (and K) features per core
    NQT = T // 512                # q-tiles of 512
    NTC = T // P                  # t-chunks of 128
    NCO = C // P                  # contraction chunks (8)
    NM = 2 * FQK // P             # Q+K feature chunks (8)
    NFC = FQK // P                # y^T feature chunks (4)
    NCT = C // 512                # output column tiles (2)
    Exp = mybir.ActivationFunctionType.Exp
    r = lambda ap: ap
    MDT = mm_dt

    nc = bacc.Bacc(target_bir_lowering=False, debug=debug)
    x = nc.dram_tensor("x", [T, C], F32, kind="ExternalInput")
    wqk = nc.dram_tensor("wqk", [C, 2 * FQK], mm_dt, kind="ExternalInput")
    wv = nc.dram_tensor("wv", [C, FQK], mm_dt, kind="ExternalInput")
    wpr = nc.dram_tensor("wpr", [FQK, C], mm_dt, kind="ExternalInput")
    bqk = nc.dram_tensor("bqk", [P, NM], F32, kind="ExternalInput")
    bv = nc.dram_tensor("bv", [P, FQK], F32, kind="ExternalInput")
    bpr = nc.dram_tensor("bpr", [P, C], F32, kind="ExternalInput")
    out = nc.dram_tensor("out", [T, C], F32, kind="ExternalOutput")

    with tile.TileContext(nc) as tc:
        with (
            tc.tile_pool(name="const", bufs=1) as cpool,
            tc.tile_pool(name="persist", bufs=1) as ppool,
            tc.tile_pool(name="xt", bufs=1) as xtp,
            tc.tile_pool(name="qt", bufs=1) as qtp,
            tc.tile_pool(name="yt", bufs=2) as ytp,
            tc.tile_pool(name="pt", bufs=3) as ptp,
            tc.tile_pool(name="yx", bufs=4) as yxp,
            tc.tile_pool(name="wqk", bufs=4) as wqkp,
            tc.tile_pool(name="xin", bufs=2) as xinp,
            tc.tile_pool(name="oout", bufs=2) as outp,
            tc.tile_pool(name="dnm", bufs=2) as dnp,
            tc.tile_pool(name="dram", bufs=3, space="DRAM") as drp,
            tc.tile_pool(name="mm", bufs=2, space="PSUM") as mmp,
            tc.tile_pool(name="sp", bufs=2, space="PSUM") as spp,
            tc.tile_pool(name="yps", bufs=2, space="PSUM") as ypp,
        ):
            # ---- constants ----
            ident = cpool.tile([P, P], F32, tag="ident")
            from concourse.masks import make_identity
            make_identity(nc, ident[:])

            # G[i, u] = 0 where u >= i + 384 else NEG; diag mask for shift s
            # is G[:, 384-s : 896-s]  (valid iff i + s <= j).
            G = cpool.tile([P, 896], F32, tag="gmask")
            nc.gpsimd.memset(G[:], 0.0)
            nc.gpsimd.affine_select(
                out=G[:], in_=G[:],
                compare_op=mybir.AluOpType.is_ge,
                fill=NEG, base=-384, channel_multiplier=-1,
                pattern=[[1, 896]],
            )

            ones_sb = cpool.tile([P, HL, 1], F32, tag="ones")
            nc.vector.memset(ones_sb[:], 1.0)
            bqk_sb = cpool.tile([P, NM], F32, tag="bqk")
            nc.sync.dma_start(bqk_sb[:], bqk[:, :])
            bv_sb = cpool.tile([P, FQK], F32, tag="bv")
            nc.sync.dma_start(bv_sb[:], bv[:, :])
            bpr_sb = cpool.tile([P, C], F32, tag="bpr")
            nc.sync.dma_start(bpr_sb[:], bpr[:, :])

            # ---- persistent tensors ----
            KT = ppool.tile([P, NFC, T], MDT, tag="KT")
            VE = ppool.tile([P, NTC, HL * (D + 1)], MDT, tag="VE")
            wv_sb = ppool.tile([P, NCO, FQK], MDT, tag="wv")
            wpr_sb = ppool.tile([P, NFC, C], MDT, tag="wpr")

            for rep in range(reps):
              for qt in range(NQT):
                q0 = qt * 512
                # ---- phase A: transpose this q-tile of x into xT ----
                xTt = xtp.tile([P, NCO, 512], MDT, tag="xT")
                for tc_i in range(4):
                    xrow = xinp.tile([P, C], F32, tag="xin")
                    nc.sync.dma_start(
                        xrow[:], x[q0 + tc_i * P: q0 + (tc_i + 1) * P, :])
                    for co in range(NCO):
                        tp = mmp.tile([P, P], F32, tag="mm")
                        nc.tensor.transpose(
                            r(tp[:]), r(xrow[:, co * P:(co + 1) * P]),
                            r(ident[:]))
                        dst = xTt[:, co, tc_i * P:(tc_i + 1) * P]
                        if co % 2 == 0:
                            nc.vector.tensor_copy(dst, tp[:])
                        else:
                            nc.scalar.copy(dst, tp[:])

                # ---- phase B: QKV projections for this q-tile ----
                QTt = qtp.tile([P, NFC, 512], MDT, tag="QTt")
                for m in range(NM):
                    ps = mmp.tile([P, 512], F32, tag="mm")
                    wt = wqkp.tile([P, NCO, P], MDT, tag="wqk")
                    nc.sync.dma_start(
                        wt[:], wqk[:, m * P:(m + 1) * P].rearrange(
                            "(co ci) f -> ci co f", ci=P))
                    for co in range(NCO):
                        nc.tensor.matmul(ps[:], r(wt[:, co, :]),
                                         r(xTt[:, co, :]),
                                         start=(co == 0), stop=(co == NCO - 1))
                    if m < NFC:
                        dst = QTt[:, m, :]
                    else:
                        dst = KT[:, m - NFC, q0:q0 + 512]
                    # dst = ps + b  (per-partition bias)
                    nc.scalar.add(dst, ps[:], bqk_sb[:, m:m + 1])

                if rep == 0 and qt == 0:
                    nc.sync.dma_start(
                        wv_sb[:], wv.rearrange("(co ci) n -> ci co n", ci=P))
                    nc.sync.dma_start(
                        wpr_sb[:], wpr.rearrange("(fo fi) n -> fi fo n",
                                                 fi=P))
                for tc_i in range(4):
                    ps = mmp.tile([P, 512], F32, tag="mm")
                    for co in range(NCO):
                        nc.tensor.matmul(
                            ps[:], r(xTt[:, co, tc_i * P:(tc_i + 1) * P]),
                            r(wv_sb[:, co, :]),
                            start=(co == 0), stop=(co == NCO - 1))
                    tci = qt * 4 + tc_i
                    vev = VE[:, tci, :].rearrange("p (h e) -> p h e", e=D + 1)
                    nc.vector.tensor_add(
                        vev[:, :, :D],
                        ps[:].rearrange("p (h d) -> p h d", d=D),
                        bv_sb[:].rearrange("p (h d) -> p h d", d=D))
                    nc.vector.tensor_copy(vev[:, :, D:D + 1], ones_sb[:])

                # ---- phase C: attention for this q-tile (head pairs) ----
                nk = 4 * (qt + 1)
                yTt = ytp.tile([P, NFC, 512], MDT, tag="yTt")
                for ch in range(NFC):            # head pair (2ch, 2ch+1)
                    ypsA = ypp.tile([P, 512], F32, tag="yps")
                    ypsB = ypp.tile([P, 512], F32, tag="yps")
                    for kc in range(nk):
                        # S^T for both heads of the pair, packed in the PE
                        # array via row tiling (each head contracts over 64
                        # partitions), into one 2-bank psum tile.
                        sp2 = spp.tile([P, 1024], F32, tag="sp")
                        nc.tensor.matmul(
                            sp2[:, 0:512],
                            r(KT[0:64, ch, kc * P:(kc + 1) * P]),
                            r(QTt[0:64, ch, :]),
                            start=True, stop=True, tile_position=(0, 0))
                        nc.tensor.matmul(
                            sp2[:, 512:1024],
                            r(KT[64:128, ch, kc * P:(kc + 1) * P]),
                            r(QTt[64:128, ch, :]),
                            start=True, stop=True, tile_position=(64, 0))
                        pt_t = ptp.tile([P, 1024], MDT, tag="pt")
                        if kc >= 4 * qt:
                            s = 128 * kc - 512 * qt
                            if gsel:
                                # exp everything, then zero the invalid
                                # (k > q) region on the idle GPSIMD engine
                                nc.scalar.activation(pt_t[:], sp2[:], Exp,
                                                     scale=0.125)
                                nc.gpsimd.affine_select(
                                    out=pt_t[:].rearrange(
                                        "p (h q) -> p h q", h=2),
                                    in_=pt_t[:].rearrange(
                                        "p (h q) -> p h q", h=2),
                                    compare_op=mybir.AluOpType.is_ge,
                                    fill=0.0, base=-s, channel_multiplier=-1,
                                    pattern=[[0, 2], [1, 512]])
                            else:
                                nc.vector.scalar_tensor_tensor(
                                    pt_t[:].rearrange("p (h q) -> p h q", h=2),
                                    sp2[:].rearrange("p (h q) -> p h q", h=2),
                                    0.125,
                                    G[:, None, 384 - s:896 - s].to_broadcast(
                                        (P, 2, 512)),
                                    mybir.AluOpType.mult,
                                    mybir.AluOpType.add)
                                nc.scalar.activation(pt_t[:], pt_t[:], Exp)
                        else:
                            nc.scalar.activation(pt_t[:], sp2[:], Exp,
                                                 scale=0.125)
                        hA, hB = 2 * ch, 2 * ch + 1
                        nc.tensor.matmul(
                            ypsA[:D + 1, :],
                            r(VE[:, kc, hA * (D + 1):(hA + 1) * (D + 1)]),
                            r(pt_t[:, 0:512]),
                            start=(kc == 0), stop=(kc == nk - 1))
                        nc.tensor.matmul(
                            ypsB[:D + 1, :],
                            r(VE[:, kc, hB * (D + 1):(hB + 1) * (D + 1)]),
                            r(pt_t[:, 512:1024]),
                            start=(kc == 0), stop=(kc == nk - 1))
                    for po, yps in ((0, ypsA), (64, ypsB)):
                        yext = yxp.tile([D + 1, 512], F32, tag="yext")
                        nc.vector.tensor_copy(yext[:], yps[:D + 1, :])
                        rd = dnp.tile([1, 512], F32, tag="rd")
                        nc.vector.reciprocal(rd[:], yext[D:D + 1, :])
                        rep = dnp.tile([64, 512], F32, tag="rep")
                        nc.gpsimd.partition_broadcast(rep[:], rd[:])
                        nc.vector.tensor_mul(
                            yTt[po:po + 64, ch, :], yext[:D, :], rep[:])

                # ---- phase D: output projection for this q-tile ----
                for tc_i in range(4):
                    for ct in range(NCT):
                        ps = mmp.tile([P, 512], F32, tag="mm")
                        for fc in range(NFC):
                            nc.tensor.matmul(
                                ps[:],
                                r(yTt[:, fc, tc_i * P:(tc_i + 1) * P]),
                                r(wpr_sb[:, fc, ct * 512:(ct + 1) * 512]),
                                start=(fc == 0), stop=(fc == NFC - 1))
                        ot = outp.tile([P, 512], F32, tag="oout")
                        nc.vector.tensor_add(
                            ot[:], ps[:], bpr_sb[:, ct * 512:(ct + 1) * 512])
                        nc.sync.dma_start(
                            out[q0 + tc_i * P:q0 + (tc_i + 1) * P,
                                ct * 512:(ct + 1) * 512], ot[:])

    nc.compile()
    return nc


_CACHE = {}


def _get_nc():
    if "nc" not in _CACHE:
        _CACHE["nc"] = build_nc()
    return _CACHE["nc"]


def make_in_maps(x, W_attn, b_attn, W_proj, b_proj, B=4, C=1024):
    x = np.ascontiguousarray(np.asarray(x, dtype=np.float32))
    W_attn = np.asarray(W_attn, dtype=np.float32)
    b_attn = np.asarray(b_attn, dtype=np.float32)
    W_proj = np.asarray(W_proj, dtype=np.float32)
    b_proj = np.asarray(b_proj, dtype=np.float32)
    in_maps = []
    for core in range(2 * B):
        b, hg = core // 2, core % 2
        s = slice(hg * 512, (hg + 1) * 512)
        wqk_c = np.ascontiguousarray(
            np.concatenate([W_attn[:, s], W_attn[:, C + hg * 512:
                                                 C + (hg + 1) * 512]], axis=1))
        wv_c = np.ascontiguousarray(W_attn[:, 2 * C + hg * 512:
                                           2 * C + (hg + 1) * 512])
        wpr_c = np.ascontiguousarray(W_proj[hg * 512:(hg + 1) * 512, :])
        bqk_vec = np.concatenate([b_attn[s], b_attn[C + hg * 512:
                                                    C + (hg + 1) * 512]])
        bqk_c = np.ascontiguousarray(bqk_vec.reshape(8, 128).T)
        bv_c = np.ascontiguousarray(
            np.tile(b_attn[2 * C + hg * 512:2 * C + (hg + 1) * 512][None, :],
                    (128, 1)))
        if hg == 0:
            bpr_c = np.ascontiguousarray(np.tile(b_proj[None, :], (128, 1)))
        else:
            bpr_c = np.zeros((128, C), dtype=np.float32)
        in_maps.append({
            "x": np.ascontiguousarray(x[b]),
            "wqk": wqk_c, "wv": wv_c, "wpr": wpr_c,
            "bqk": bqk_c, "bv": bv_c, "bpr": bpr_c,
        })
    return in_maps


def kernel(x, W_attn, b_attn, W_proj, b_proj):
    B, T, C = 4, 2048, 1024
    nc = _get_nc()
    in_maps = make_in_maps(x, W_attn, b_attn, W_proj, b_proj, B=B, C=C)
    res = run_bass_kernel_spmd(nc, in_maps, list(range(2 * B)))
    out = np.empty((B, T, C), dtype=np.float32)
    for b in range(B):
        out[b] = res.results[2 * b]["out"] + res.results[2 * b + 1]["out"]
    return out


# revision 13
# speedup vs baseline: 1.3039x; 1.0982x over previous
"""Causal self-attention Trainium2 kernel (8-core SPMD).

Sharding: 8 cores = 4 batches x 2 head-groups (tensor parallel over heads).
Each core computes, for its batch b and its 8 heads:
  QKV projection (transposed layouts), causal flash-style attention without
  max-subtraction (scores are O(+-10), safe in fp32), and a partial output
  projection over its head-group's rows of W_proj.  The host sums the two
  partial outputs per batch (the "all-reduce" of the hint, done host-side).

Device layouts (per core):
  x       [T, C]    this batch's activations
  xT      [C, T]    built on-device via PE transposes (fp32 has no DMA xpose)
  Q^T,K^T [f, t]    f = head-major features (head pair per 128-chunk)
  V_ext   [t, 8*65] per head: 64 V columns + a ones column (softmax denom
                    falls out of the attn@V matmul for free)
  S^T     [k, q]    scores transposed; softmax denom = ones-row of V_ext
  y^T     [f, t]    normalized attention output, feeds W_proj matmul
  out     [T, C]    partial projection output (host adds the two halves)
"""

import numpy as np

import concourse.bass as bass
import concourse.mybir as mybir
import concourse.tile as tile
from concourse import bacc
from concourse.bass_utils import run_bass_kernel_spmd

F32 = mybir.dt.float32
P = 128
NEG = -1.0e30


def build_nc(T=2048, C=1024, n_loc_heads=8, debug=False, reps=1,
             mm_dt=mybir.dt.float32r, gsel=True):
    """Build the per-core SPMD program. T must be a multiple of 512."""
    D = 64
    HL = n_loc_heads              # local heads (8)
    FQK = HL * D                  # 512: Q (and K) features per core
    NQT = T // 512                # q-tiles of 512
    NTC = T // P                  # t-chunks of 128
    NCO = C // P                  # contraction chunks (8)
    NM = 2 * FQK // P             # Q+K feature chunks (8)
    NFC = FQK // P                # y^T feature chunks (4)
    NCT = C // 512                # output column tiles (2)
    Exp = mybir.ActivationFunctionType.Exp
    r = lambda ap: ap
    MDT = mm_dt

    nc = bacc.Bacc(target_bir_lowering=False, debug=debug)
    x = nc.dram_tensor("x", [T, C], F32, kind="ExternalInput")
    wqk = nc.dram_tensor("wqk", [2 * FQK // P, P, C // P, P], mm_dt,
                     kind="ExternalInput")
    wv = nc.dram_tensor("wv", [C, FQK], mm_dt, kind="ExternalInput")
    wpr = nc.dram_tensor("wpr", [FQK, C], mm_dt, kind="ExternalInput")
    bqk = nc.dram_tensor("bqk", [P, NM], F32, kind="ExternalInput")
    bv = nc.dram_tensor("bv", [P, FQK], F32, kind="ExternalInput")
    bpr = nc.dram_tensor("bpr", [P, C], F32, kind="ExternalInput")
    out = nc.dram_tensor("out", [T, C], F32, kind="ExternalOutput")

    with tile.TileContext(nc) as tc:
        with (
            tc.tile_pool(name="const", bufs=1) as cpool,
            tc.tile_pool(name="persist", bufs=1) as ppool,
            tc.tile_pool(name="xt", bufs=1) as xtp,
            tc.tile_pool(name="qt", bufs=1) as qtp,
            tc.tile_pool(name="yt", bufs=2) as ytp,
            tc.tile_pool(name="pt", bufs=3) as ptp,
            tc.tile_pool(name="yx", bufs=4) as yxp,
            tc.tile_pool(name="wqk", bufs=4) as wqkp,
            tc.tile_pool(name="xin", bufs=2) as xinp,
            tc.tile_pool(name="oout", bufs=2) as outp,
            tc.tile_pool(name="dnm", bufs=2) as dnp,
            tc.tile_pool(name="dram", bufs=3, space="DRAM") as drp,
            tc.tile_pool(name="mm", bufs=2, space="PSUM") as mmp,
            tc.tile_pool(name="sp", bufs=2, space="PSUM") as spp,
            tc.tile_pool(name="yps", bufs=2, space="PSUM") as ypp,
        ):
            # ---- constants ----
            ident = cpool.tile([P, P], F32, tag="ident")
            from concourse.masks import make_identity
            make_identity(nc, ident[:])

            # G[i, u] = 0 where u >= i + 384 else NEG; diag mask for shift s
            # is G[:, 384-s : 896-s]  (valid iff i + s <= j).
            G = cpool.tile([P, 896], F32, tag="gmask")
            nc.gpsimd.memset(G[:], 0.0)
            nc.gpsimd.affine_select(
                out=G[:], in_=G[:],
                compare_op=mybir.AluOpType.is_ge,
                fill=NEG, base=-384, channel_multiplier=-1,
                pattern=[[1, 896]],
            )

            ones_sb = cpool.tile([P, HL, 1], F32, tag="ones")
            nc.vector.memset(ones_sb[:], 1.0)
            bqk_sb = cpool.tile([P, NM], F32, tag="bqk")
            nc.sync.dma_start(bqk_sb[:], bqk[:, :])
            bv_sb = cpool.tile([P, FQK], F32, tag="bv")
            nc.sync.dma_start(bv_sb[:], bv[:, :])
            bpr_sb = cpool.tile([P, C], F32, tag="bpr")
            nc.sync.dma_start(bpr_sb[:], bpr[:, :])

            # ---- persistent tensors ----
            KT = ppool.tile([P, NFC, T], MDT, tag="KT")
            VE = ppool.tile([P, NTC, HL * (D + 1)], MDT, tag="VE")
            wv_sb = ppool.tile([P, NCO, FQK], MDT, tag="wv")
            wpr_sb = ppool.tile([P, NFC, C], MDT, tag="wpr")

            for rep in range(reps):
              for qt in range(NQT):
                q0 = qt * 512
                # ---- phase A: transpose this q-tile of x into xT ----
                xTt = xtp.tile([P, NCO, 512], MDT, tag="xT")
                for tc_i in range(4):
                    xrow = xinp.tile([P, C], F32, tag="xin")
                    nc.sync.dma_start(
                        xrow[:], x[q0 + tc_i * P: q0 + (tc_i + 1) * P, :])
                    for co in range(NCO):
                        tp = mmp.tile([P, P], F32, tag="mm")
                        nc.tensor.transpose(
                            r(tp[:]), r(xrow[:, co * P:(co + 1) * P]),
                            r(ident[:]))
                        dst = xTt[:, co, tc_i * P:(tc_i + 1) * P]
                        if co % 2 == 0:
                            nc.vector.tensor_copy(dst, tp[:])
                        else:
                            nc.scalar.copy(dst, tp[:])

                # ---- phase B: QKV projections for this q-tile ----
                QTt = qtp.tile([P, NFC, 512], MDT, tag="QTt")
                for m in range(NM):
                    ps = mmp.tile([P, 512], F32, tag="mm")
                    wt = wqkp.tile([P, NCO, P], MDT, tag="wqk")
                    nc.sync.dma_start(wt[:], wqk[m])
                    for co in range(NCO):
                        nc.tensor.matmul(ps[:], r(wt[:, co, :]),
                                         r(xTt[:, co, :]),
                                         start=(co == 0), stop=(co == NCO - 1))
                    if m < NFC:
                        dst = QTt[:, m, :]
                    else:
                        dst = KT[:, m - NFC, q0:q0 + 512]
                    # dst = ps + b  (per-partition bias)
                    nc.scalar.add(dst, ps[:], bqk_sb[:, m:m + 1])

                if rep == 0 and qt == 0:
                    nc.sync.dma_start(
                        wv_sb[:], wv.rearrange("(co ci) n -> ci co n", ci=P))
                    nc.sync.dma_start(
                        wpr_sb[:], wpr.rearrange("(fo fi) n -> fi fo n",
                                                 fi=P))
                for tc_i in range(4):
                    ps = mmp.tile([P, 512], F32, tag="mm")
                    for co in range(NCO):
                        nc.tensor.matmul(
                            ps[:], r(xTt[:, co, tc_i * P:(tc_i + 1) * P]),
                            r(wv_sb[:, co, :]),
                            start=(co == 0), stop=(co == NCO - 1))
                    tci = qt * 4 + tc_i
                    vev = VE[:, tci, :].rearrange("p (h e) -> p h e", e=D + 1)
                    nc.vector.tensor_add(
                        vev[:, :, :D],
                        ps[:].rearrange("p (h d) -> p h d", d=D),
                        bv_sb[:].rearrange("p (h d) -> p h d", d=D))
                    nc.vector.tensor_copy(vev[:, :, D:D + 1], ones_sb[:])

                # ---- phase C: attention for this q-tile (head pairs) ----
                nk = 4 * (qt + 1)
                yTt = ytp.tile([P, NFC, 512], MDT, tag="yTt")
                for ch in range(NFC):            # head pair (2ch, 2ch+1)
                    ypsA = ypp.tile([P, 512], F32, tag="yps")
                    ypsB = ypp.tile([P, 512], F32, tag="yps")
                    for kc in range(nk):
                        # S^T for both heads of the pair, packed in the PE
                        # array via row tiling (each head contracts over 64
                        # partitions), into one 2-bank psum tile.
                        sp2 = spp.tile([P, 1024], F32, tag="sp")
                        nc.tensor.matmul(
                            sp2[:, 0:512],
                            r(KT[0:64, ch, kc * P:(kc + 1) * P]),
                            r(QTt[0:64, ch, :]),
                            start=True, stop=True, tile_position=(0, 0))
                        nc.tensor.matmul(
                            sp2[:, 512:1024],
                            r(KT[64:128, ch, kc * P:(kc + 1) * P]),
                            r(QTt[64:128, ch, :]),
                            start=True, stop=True, tile_position=(64, 0))
                        pt_t = ptp.tile([P, 1024], MDT, tag="pt")
                        if kc >= 4 * qt:
                            s = 128 * kc - 512 * qt
                            if gsel:
                                # exp everything, then zero the invalid
                                # (k > q) region on the idle GPSIMD engine
                                nc.scalar.activation(pt_t[:], sp2[:], Exp,
                                                     scale=0.125)
                                nc.gpsimd.affine_select(
                                    out=pt_t[:].rearrange(
                                        "p (h q) -> p h q", h=2),
                                    in_=pt_t[:].rearrange(
                                        "p (h q) -> p h q", h=2),
                                    compare_op=mybir.AluOpType.is_ge,
                                    fill=0.0, base=-s, channel_multiplier=-1,
                                    pattern=[[0, 2], [1, 512]])
                            else:
                                nc.vector.scalar_tensor_tensor(
                                    pt_t[:].rearrange("p (h q) -> p h q", h=2),
                                    sp2[:].rearrange("p (h q) -> p h q", h=2),
                                    0.125,
                                    G[:, None, 384 - s:896 - s].to_broadcast(
                                        (P, 2, 512)),
                                    mybir.AluOpType.mult,
                                    mybir.AluOpType.add)
                                nc.scalar.activation(pt_t[:], pt_t[:], Exp)
                        else:
                            nc.scalar.activation(pt_t[:], sp2[:], Exp,
                                                 scale=0.125)
                        hA, hB = 2 * ch, 2 * ch + 1
                        nc.tensor.matmul(
                            ypsA[:D + 1, :],
                            r(VE[:, kc, hA * (D + 1):(hA + 1) * (D + 1)]),
                            r(pt_t[:, 0:512]),
                            start=(kc == 0), stop=(kc == nk - 1))
                        nc.tensor.matmul(
                            ypsB[:D + 1, :],
                            r(VE[:, kc, hB * (D + 1):(hB + 1) * (D + 1)]),
                            r(pt_t[:, 512:1024]),
                            start=(kc == 0), stop=(kc == nk - 1))
                    for po, yps in ((0, ypsA), (64, ypsB)):
                        yext = yxp.tile([D + 1, 512], F32, tag="yext")
                        nc.vector.tensor_copy(yext[:], yps[:D + 1, :])
                        rd = dnp.tile([1, 512], F32, tag="rd")
                        nc.vector.reciprocal(rd[:], yext[D:D + 1, :])
                        rep = dnp.tile([64, 512], F32, tag="rep")
                        nc.gpsimd.partition_broadcast(rep[:], rd[:])
                        nc.vector.tensor_mul(
                            yTt[po:po + 64, ch, :], yext[:D, :], rep[:])

                # ---- phase D: output projection for this q-tile ----
                for tc_i in range(4):
                    for ct in range(NCT):
                        ps = mmp.tile([P, 512], F32, tag="mm")
                        for fc in range(NFC):
                            nc.tensor.matmul(
                                ps[:],
                                r(yTt[:, fc, tc_i * P:(tc_i + 1) * P]),
                                r(wpr_sb[:, fc, ct * 512:(ct + 1) * 512]),
                                start=(fc == 0), stop=(fc == NFC - 1))
                        ot = outp.tile([P, 512], F32, tag="oout")
                        nc.vector.tensor_add(
                            ot[:], ps[:], bpr_sb[:, ct * 512:(ct + 1) * 512])
                        nc.sync.dma_start(
                            out[q0 + tc_i * P:q0 + (tc_i + 1) * P,
                                ct * 512:(ct + 1) * 512], ot[:])

    nc.compile()
    return nc


_CACHE = {}


def _get_nc():
    if "nc" not in _CACHE:
        _CACHE["nc"] = build_nc()
    return _CACHE["nc"]


def make_in_maps(x, W_attn, b_attn, W_proj, b_proj, B=4, C=1024):
    x = np.ascontiguousarray(np.asarray(x, dtype=np.float32))
    W_attn = np.asarray(W_attn, dtype=np.float32)
    b_attn = np.asarray(b_attn, dtype=np.float32)
    W_proj = np.asarray(W_proj, dtype=np.float32)
    b_proj = np.asarray(b_proj, dtype=np.float32)
    in_maps = []
    for core in range(2 * B):
        b, hg = core // 2, core % 2
        s = slice(hg * 512, (hg + 1) * 512)
        wqk_flat = np.concatenate(
            [W_attn[:, s], W_attn[:, C + hg * 512:C + (hg + 1) * 512]],
            axis=1)  # [C, 1024]
        # [m, ci, co, f]: per m-chunk one contiguous [128, 8, 128] block
        # (4KB per partition row -> large DMA descriptors).
        # wqk_flat[co*128+ci, m*128+f] -> reshape [co, ci, m, f]
        wqk_c = np.ascontiguousarray(
            wqk_flat.reshape(8, 128, 8, 128).transpose(2, 1, 0, 3))
        wv_c = np.ascontiguousarray(W_attn[:, 2 * C + hg * 512:
                                           2 * C + (hg + 1) * 512])
        wpr_c = np.ascontiguousarray(W_proj[hg * 512:(hg + 1) * 512, :])
        bqk_vec = np.concatenate([b_attn[s], b_attn[C + hg * 512:
                                                    C + (hg + 1) * 512]])
        bqk_c = np.ascontiguousarray(bqk_vec.reshape(8, 128).T)
        bv_c = np.ascontiguousarray(
            np.tile(b_attn[2 * C + hg * 512:2 * C + (hg + 1) * 512][None, :],
                    (128, 1)))
        if hg == 0:
            bpr_c = np.ascontiguousarray(np.tile(b_proj[None, :], (128, 1)))
        else:
            bpr_c = np.zeros((128, C), dtype=np.float32)
        in_maps.append({
            "x": np.ascontiguousarray(x[b]),
            "wqk": wqk_c, "wv": wv_c, "wpr": wpr_c,
            "bqk": bqk_c, "bv": bv_c, "bpr": bpr_c,
        })
    return in_maps


def kernel(x, W_attn, b_attn, W_proj, b_proj):
    B, T, C = 4, 2048, 1024
    nc = _get_nc()
    in_maps = make_in_maps(x, W_attn, b_attn, W_proj, b_proj, B=B, C=C)
    res = run_bass_kernel_spmd(nc, in_maps, list(range(2 * B)))
    out = np.empty((B, T, C), dtype=np.float32)
    for b in range(B):
        out[b] = res.results[2 * b]["out"] + res.results[2 * b + 1]["out"]
    return out


# revision 14
# speedup vs baseline: 3.3265x; 2.5512x over previous
"""Causal self-attention Trainium2 kernel (8-core SPMD).

Sharding: 8 cores = 4 batches x 2 head-groups (tensor parallel over heads).
Each core computes, for its batch b and its 8 heads:
  QKV projection (transposed layouts), causal flash-style attention without
  max-subtraction (scores are O(+-10), safe in fp32), and a partial output
  projection over its head-group's rows of W_proj.  The host sums the two
  partial outputs per batch (the "all-reduce" of the hint, done host-side).

Device layouts (per core):
  x       [T, C]    this batch's activations
  xT      [C, T]    built on-device via PE transposes (fp32 has no DMA xpose)
  Q^T,K^T [f, t]    f = head-major features (head pair per 128-chunk)
  V_ext   [t, 8*65] per head: 64 V columns + a ones column (softmax denom
                    falls out of the attn@V matmul for free)
  S^T     [k, q]    scores transposed; softmax denom = ones-row of V_ext
  y^T     [f, t]    normalized attention output, feeds W_proj matmul
  out     [T, C]    partial projection output (host adds the two halves)
"""

import numpy as np

import concourse.bass as bass
import concourse.mybir as mybir
import concourse.tile as tile
from concourse import bacc
from concourse.bass_utils import run_bass_kernel_spmd

F32 = mybir.dt.float32
P = 128
NEG = -1.0e30


def build_nc(T=2048, C=1024, n_loc_heads=8, debug=False, reps=1,
             mm_dt=mybir.dt.float32r, gsel=True):
    """Build the per-core SPMD program. T must be a multiple of 512."""
    D = 64
    HL = n_loc_heads              # local heads (8)
    FQK = HL * D                  # 512: Q (and K) features per core
    NQT = T // 512                # q-tiles of 512
    NTC = T // P                  # t-chunks of 128
    NCO = C // P                  # contraction chunks (8)
    NM = 2 * FQK // P             # Q+K feature chunks (8)
    NFC = FQK // P                # y^T feature chunks (4)
    NCT = C // 512                # output column tiles (2)
    Exp = mybir.ActivationFunctionType.Exp
    r = lambda ap: ap
    MDT = mm_dt

    nc = bacc.Bacc(target_bir_lowering=False, debug=debug)
    x = nc.dram_tensor("x", [T, C], F32, kind="ExternalInput")
    wqk = nc.dram_tensor("wqk", [2 * FQK // P, P, C // P, P], mm_dt,
                     kind="ExternalInput")
    wv = nc.dram_tensor("wv", [C, FQK], mm_dt, kind="ExternalInput")
    wpr = nc.dram_tensor("wpr", [FQK, C], mm_dt, kind="ExternalInput")
    bqk = nc.dram_tensor("bqk", [P, NM], F32, kind="ExternalInput")
    bv = nc.dram_tensor("bv", [P, FQK], F32, kind="ExternalInput")
    bpr = nc.dram_tensor("bpr", [P, C], F32, kind="ExternalInput")
    out = nc.dram_tensor("out", [T, C], F32, kind="ExternalOutput")

    with tile.TileContext(nc) as tc:
        with (
            tc.tile_pool(name="const", bufs=1) as cpool,
            tc.tile_pool(name="persist", bufs=1) as ppool,
            tc.tile_pool(name="xt", bufs=1) as xtp,
            tc.tile_pool(name="qt", bufs=1) as qtp,
            tc.tile_pool(name="yt", bufs=2) as ytp,
            tc.tile_pool(name="pt", bufs=3) as ptp,
            tc.tile_pool(name="yx", bufs=4) as yxp,
            tc.tile_pool(name="wqk", bufs=4) as wqkp,
            tc.tile_pool(name="xin", bufs=2) as xinp,
            tc.tile_pool(name="oout", bufs=2) as outp,
            tc.tile_pool(name="dnm", bufs=2) as dnp,
            tc.tile_pool(name="mm", bufs=2, space="PSUM") as mmp,
            tc.tile_pool(name="sp", bufs=2, space="PSUM") as spp,
            tc.tile_pool(name="yps", bufs=2, space="PSUM") as ypp,
        ):
            # ---- constants ----
            ident = cpool.tile([P, P], F32, tag="ident")
            from concourse.masks import make_identity
            make_identity(nc, ident[:])

            # G[i, u] = 0 where u >= i + 384 else NEG; diag mask for shift s
            # is G[:, 384-s : 896-s]  (valid iff i + s <= j).
            G = cpool.tile([P, 896], F32, tag="gmask")
            nc.gpsimd.memset(G[:], 0.0)
            nc.gpsimd.affine_select(
                out=G[:], in_=G[:],
                compare_op=mybir.AluOpType.is_ge,
                fill=NEG, base=-384, channel_multiplier=-1,
                pattern=[[1, 896]],
            )

            ones_sb = cpool.tile([P, HL, 1], F32, tag="ones")
            nc.vector.memset(ones_sb[:], 1.0)
            bqk_sb = cpool.tile([P, NM], F32, tag="bqk")
            nc.sync.dma_start(bqk_sb[:], bqk[:, :])
            bv_sb = cpool.tile([P, FQK], F32, tag="bv")
            nc.sync.dma_start(bv_sb[:], bv[:, :])
            bpr_sb = cpool.tile([P, C], F32, tag="bpr")
            nc.sync.dma_start(bpr_sb[:], bpr[:, :])

            # ---- persistent tensors ----
            KT = ppool.tile([P, NFC, T], MDT, tag="KT")
            VE = ppool.tile([P, NTC, HL * (D + 1)], MDT, tag="VE")
            wv_sb = ppool.tile([P, NCO, FQK], MDT, tag="wv")
            wpr_sb = ppool.tile([P, NFC, C], MDT, tag="wpr")

            for rep in range(reps):
              for qt in range(NQT):
                q0 = qt * 512
                # ---- phase A: transpose this q-tile of x into xT ----
                xTt = xtp.tile([P, NCO, 512], MDT, tag="xT")
                for tc_i in range(4):
                    xrow = xinp.tile([P, C], F32, tag="xin")
                    nc.sync.dma_start(
                        xrow[:], x[q0 + tc_i * P: q0 + (tc_i + 1) * P, :])
                    for co in range(NCO):
                        tp = mmp.tile([P, P], F32, tag="mm")
                        nc.tensor.transpose(
                            r(tp[:]), r(xrow[:, co * P:(co + 1) * P]),
                            r(ident[:]))
                        dst = xTt[:, co, tc_i * P:(tc_i + 1) * P]
                        if co % 2 == 0:
                            nc.vector.tensor_copy(dst, tp[:])
                        else:
                            nc.scalar.copy(dst, tp[:])

                # ---- phase B: QKV projections for this q-tile ----
                QTt = qtp.tile([P, NFC, 512], MDT, tag="QTt")
                for m in range(NM):
                    ps = mmp.tile([P, 512], F32, tag="mm")
                    wt = wqkp.tile([P, NCO, P], MDT, tag="wqk")
                    nc.sync.dma_start(wt[:], wqk[m])
                    for co in range(NCO):
                        nc.tensor.matmul(ps[:], r(wt[:, co, :]),
                                         r(xTt[:, co, :]),
                                         start=(co == 0), stop=(co == NCO - 1))
                    if m < NFC:
                        dst = QTt[:, m, :]
                    else:
                        dst = KT[:, m - NFC, q0:q0 + 512]
                    # dst = ps + b  (per-partition bias)
                    nc.scalar.add(dst, ps[:], bqk_sb[:, m:m + 1])

                if rep == 0 and qt == 0:
                    nc.sync.dma_start(
                        wv_sb[:], wv.rearrange("(co ci) n -> ci co n", ci=P))
                    nc.sync.dma_start(
                        wpr_sb[:], wpr.rearrange("(fo fi) n -> fi fo n",
                                                 fi=P))
                for tc_i in range(4):
                    ps = mmp.tile([P, 512], F32, tag="mm")
                    for co in range(NCO):
                        nc.tensor.matmul(
                            ps[:], r(xTt[:, co, tc_i * P:(tc_i + 1) * P]),
                            r(wv_sb[:, co, :]),
                            start=(co == 0), stop=(co == NCO - 1))
                    tci = qt * 4 + tc_i
                    vev = VE[:, tci, :].rearrange("p (h e) -> p h e", e=D + 1)
                    nc.vector.tensor_add(
                        vev[:, :, :D],
                        ps[:].rearrange("p (h d) -> p h d", d=D),
                        bv_sb[:].rearrange("p (h d) -> p h d", d=D))
                    nc.vector.tensor_copy(vev[:, :, D:D + 1], ones_sb[:])

                # ---- phase C: attention for this q-tile (head pairs) ----
                nk = 4 * (qt + 1)
                yTt = ytp.tile([P, NFC, 512], MDT, tag="yTt")
                for ch in range(NFC):            # head pair (2ch, 2ch+1)
                    ypsA = ypp.tile([P, 512], F32, tag="yps")
                    ypsB = ypp.tile([P, 512], F32, tag="yps")
                    for kc in range(nk):
                        # S^T for both heads of the pair, packed in the PE
                        # array via row tiling (each head contracts over 64
                        # partitions), into one 2-bank psum tile.
                        sp2 = spp.tile([P, 1024], F32, tag="sp")
                        nc.tensor.matmul(
                            sp2[:, 0:512],
                            r(KT[0:64, ch, kc * P:(kc + 1) * P]),
                            r(QTt[0:64, ch, :]),
                            start=True, stop=True, tile_position=(0, 0))
                        nc.tensor.matmul(
                            sp2[:, 512:1024],
                            r(KT[64:128, ch, kc * P:(kc + 1) * P]),
                            r(QTt[64:128, ch, :]),
                            start=True, stop=True, tile_position=(64, 0))
                        pt_t = ptp.tile([P, 1024], MDT, tag="pt")
                        if kc >= 4 * qt:
                            s = 128 * kc - 512 * qt
                            if gsel:
                                # exp everything, then zero the invalid
                                # (k > q) region on the idle GPSIMD engine
                                nc.scalar.activation(pt_t[:], sp2[:], Exp,
                                                     scale=0.125)
                                nc.gpsimd.affine_select(
                                    out=pt_t[:].rearrange(
                                        "p (h q) -> p h q", h=2),
                                    in_=pt_t[:].rearrange(
                                        "p (h q) -> p h q", h=2),
                                    compare_op=mybir.AluOpType.is_ge,
                                    fill=0.0, base=-s, channel_multiplier=-1,
                                    pattern=[[0, 2], [1, 512]])
                            else:
                                nc.vector.scalar_tensor_tensor(
                                    pt_t[:].rearrange("p (h q) -> p h q", h=2),
                                    sp2[:].rearrange("p (h q) -> p h q", h=2),
                                    0.125,
                                    G[:, None, 384 - s:896 - s].to_broadcast(
                                        (P, 2, 512)),
                                    mybir.AluOpType.mult,
                                    mybir.AluOpType.add)
                                nc.scalar.activation(pt_t[:], pt_t[:], Exp)
                        else:
                            nc.scalar.activation(pt_t[:], sp2[:], Exp,
                                                 scale=0.125)
                        hA, hB = 2 * ch, 2 * ch + 1
                        nc.tensor.matmul(
                            ypsA[:D + 1, :],
                            r(VE[:, kc, hA * (D + 1):(hA + 1) * (D + 1)]),
                            r(pt_t[:, 0:512]),
                            start=(kc == 0), stop=(kc == nk - 1))
                        nc.tensor.matmul(
                            ypsB[:D + 1, :],
                            r(VE[:, kc, hB * (D + 1):(hB + 1) * (D + 1)]),
                            r(pt_t[:, 512:1024]),
                            start=(kc == 0), stop=(kc == nk - 1))
                    for po, yps in ((0, ypsA), (64, ypsB)):
                        yext = yxp.tile([D + 1, 512], F32, tag="yext")
                        nc.vector.tensor_copy(yext[:], yps[:D + 1, :])
                        rd = dnp.tile([1, 512], F32, tag="rd")
                        nc.vector.reciprocal(rd[:], yext[D:D + 1, :])
                        repb = dnp.tile([64, 512], F32, tag="rep")
                        nc.gpsimd.partition_broadcast(repb[:], rd[:])
                        nc.vector.tensor_mul(
                            yTt[po:po + 64, ch, :], yext[:D, :], repb[:])

                # ---- phase D: output projection for this q-tile ----
                for tc_i in range(4):
                    for ct in range(NCT):
                        ps = mmp.tile([P, 512], F32, tag="mm")
                        for fc in range(NFC):
                            nc.tensor.matmul(
                                ps[:],
                                r(yTt[:, fc, tc_i * P:(tc_i + 1) * P]),
                                r(wpr_sb[:, fc, ct * 512:(ct + 1) * 512]),
                                start=(fc == 0), stop=(fc == NFC - 1))
                        ot = outp.tile([P, 512], F32, tag="oout")
                        nc.vector.tensor_add(
                            ot[:], ps[:], bpr_sb[:, ct * 512:(ct + 1) * 512])
                        nc.sync.dma_start(
                            out[q0 + tc_i * P:q0 + (tc_i + 1) * P,
                                ct * 512:(ct + 1) * 512], ot[:])

    nc.compile()
    return nc


_CACHE = {}


def _get_nc():
    if "nc" not in _CACHE:
        _CACHE["nc"] = build_nc()
    return _CACHE["nc"]


def make_in_maps(x, W_attn, b_attn, W_proj, b_proj, B=4, C=1024):
    x = np.ascontiguousarray(np.asarray(x, dtype=np.float32))
    W_attn = np.asarray(W_attn, dtype=np.float32)
    b_attn = np.asarray(b_attn, dtype=np.float32)
    W_proj = np.asarray(W_proj, dtype=np.float32)
    b_proj = np.asarray(b_proj, dtype=np.float32)
    in_maps = []
    for core in range(2 * B):
        b, hg = core // 2, core % 2
        s = slice(hg * 512, (hg + 1) * 512)
        wqk_flat = np.concatenate(
            [W_attn[:, s], W_attn[:, C + hg * 512:C + (hg + 1) * 512]],
            axis=1)  # [C, 1024]
        # [m, ci, co, f]: per m-chunk one contiguous [128, 8, 128] block
        # (4KB per partition row -> large DMA descriptors).
        # wqk_flat[co*128+ci, m*128+f] -> reshape [co, ci, m, f]
        wqk_c = np.ascontiguousarray(
            wqk_flat.reshape(8, 128, 8, 128).transpose(2, 1, 0, 3))
        wv_c = np.ascontiguousarray(W_attn[:, 2 * C + hg * 512:
                                           2 * C + (hg + 1) * 512])
        wpr_c = np.ascontiguousarray(W_proj[hg * 512:(hg + 1) * 512, :])
        bqk_vec = np.concatenate([b_attn[s], b_attn[C + hg * 512:
                                                    C + (hg + 1) * 512]])
        bqk_c = np.ascontiguousarray(bqk_vec.reshape(8, 128).T)
        bv_c = np.ascontiguousarray(
            np.tile(b_attn[2 * C + hg * 512:2 * C + (hg + 1) * 512][None, :],
                    (128, 1)))
        if hg == 0:
            bpr_c = np.ascontiguousarray(np.tile(b_proj[None, :], (128, 1)))
        else:
            bpr_c = np.zeros((128, C), dtype=np.float32)
        in_maps.append({
            "x": np.ascontiguousarray(x[b]),
            "wqk": wqk_c, "wv": wv_c, "wpr": wpr_c,
            "bqk": bqk_c, "bv": bv_c, "bpr": bpr_c,
        })
    return in_maps


def kernel(x, W_attn, b_attn, W_proj, b_proj):
    B, T, C = 4, 2048, 1024
    nc = _get_nc()
    in_maps = make_in_maps(x, W_attn, b_attn, W_proj, b_proj, B=B, C=C)
    res = run_bass_kernel_spmd(nc, in_maps, list(range(2 * B)))
    out = np.empty((B, T, C), dtype=np.float32)
    for b in range(B):
        out[b] = res.results[2 * b]["out"] + res.results[2 * b + 1]["out"]
    return out
